# revision 1
# baseline (speedup 1.0000x reference)
"""Expert-parallel grouped GEMM (MoE) kernel for Trainium2.

Problem: out[e] = gelu(tok[e] @ w1[e]) @ w2[e]  per expert e.
  tok: [128, 2048, 128] f32, w1: [128, 128, 512] f32, w2: [128, 512, 128] f32.

Sharding: expert-parallel across 8 NeuronCores, 16 experts per core, no
cross-core communication. Each core runs the same Bass program on its own
expert slice (SPMD), the host concatenates the per-core outputs.

Per-core dataflow:
  - tokens loaded naturally ([t,d] tiles), PE-transposed to [d,t] (contraction
    dim of GEMM1 must sit on partitions; fp32 has no DMA-transpose path)
  - GEMM1 in fp32r (full-rate PE, ~13-bit mantissa): hT = w1.T @ tokT
  - GELU (exact/erf) on ScalarE, PSUM -> SBUF, output rounded to fp32r
  - GEMM2 in fp32r: outT[o, t] = sum_hd w2_tile.T @ hT[hd] (PSUM accumulation)
  - PE-transpose outT back to [t, o] in fp32, DMA out naturally
"""

import numpy as np

NUM_CORES = 8
E_TOTAL = 128
E_PER_CORE = E_TOTAL // NUM_CORES  # 16
T = 2048
D = 128
H = 512
O = 128
P = 128

T_CHUNK = 512  # tokens per GEMM moving-operand chunk
N_CHUNKS = T // T_CHUNK  # 4
BLKS_PER_CHUNK = T_CHUNK // P  # 4
N_BLKS = T // P  # 16

_CACHE = {}


DEFAULT_CFG = dict(
    gelu_pair=True,  # True: one [128, 2*512] psum + one gelu per hd pair
    gelu_quad=False,
    chunked_tok=True,
    chunk_first_only=True,  # chunk-granular token load only for expert 0 (startup)
    chunked_out=True,  # per-chunk output stores (earlier store start, smaller tail)
    pt_bufs=2,
    ph_bufs=2,
    po_bufs=1,
    pot_bufs=1,
    tokt_bufs=4,
    h_bufs=8,
    osb_bufs=3,
    tokn_bufs=6,
    outn_bufs=6,
    w_bufs=2,
)


def _build(loop=1, cfg=None):
    import concourse.bacc as bacc
    import concourse.mybir as mybir
    import concourse.tile as tile
    from concourse.masks import make_identity

    f32 = mybir.dt.float32
    f32r = mybir.dt.float32r
    GELU = mybir.ActivationFunctionType.Gelu
    C = dict(DEFAULT_CFG)
    if cfg:
        C.update(cfg)

    nc = bacc.Bacc(
        "TRN2",
        target_bir_lowering=False,
        debug=False,
        num_devices=NUM_CORES,
    )

    tok = nc.dram_tensor(
        "group_token", [E_PER_CORE, T, D], f32, kind="ExternalInput"
    ).ap()
    w1 = nc.dram_tensor("weights1", [E_PER_CORE, D, H], f32, kind="ExternalInput").ap()
    w2 = nc.dram_tensor("weights2", [E_PER_CORE, H, O], f32, kind="ExternalInput").ap()
    out = nc.dram_tensor("out", [E_PER_CORE, T, O], f32, kind="ExternalOutput").ap()

    H_TILES = H // P  # 4

    with tile.TileContext(nc) as tc:
        with (
            tc.tile_pool(name="const", bufs=1) as const_pool,
            tc.tile_pool(name="weights", bufs=C["w_bufs"]) as w_pool,
            tc.tile_pool(name="tokn", bufs=C["tokn_bufs"]) as tokn_pool,
            tc.tile_pool(name="tokt", bufs=C["tokt_bufs"]) as tokt_pool,
            tc.tile_pool(name="hts", bufs=C["h_bufs"]) as h_pool,
            tc.tile_pool(name="osb", bufs=C["osb_bufs"]) as osb_pool,
            tc.tile_pool(name="outn", bufs=C["outn_bufs"]) as outn_pool,
            tc.tile_pool(name="pt", bufs=C["pt_bufs"], space="PSUM") as pt_pool,
            tc.tile_pool(name="ph", bufs=C["ph_bufs"], space="PSUM") as ph_pool,
            tc.tile_pool(name="po", bufs=C["po_bufs"], space="PSUM") as po_pool,
            tc.tile_pool(name="pot", bufs=C["pot_bufs"], space="PSUM") as pot_pool,
        ):
            ident_f32 = const_pool.tile([P, P], f32)
            make_identity(nc, ident_f32)
            ident = const_pool.tile([P, P], f32r)
            nc.vector.tensor_copy(ident[:], ident_f32[:])

            def body(_iv=None):
                for e in range(E_PER_CORE):
                    # tokens: partition p holds the 16 consecutive tokens
                    # t = p*16 + m (m = 4c + j), so both the token load and the
                    # output store are 8 KiB-contiguous per partition.
                    # For expert 0 the chunk-0 token DMA is emitted before the
                    # weight DMAs: SWDGE descriptor generation is serial on the
                    # GpSimd Q7 and the transposes need tokens first.
                    chunk_this = C.get("chunked_tok") and (
                        e == 0 or not C.get("chunk_first_only")
                    )
                    tokn_chunks = None
                    if chunk_this:
                        tokn_chunks = []
                        for cc in range(N_CHUNKS):
                            tkc = tokn_pool.tile(
                                [P, BLKS_PER_CHUNK, D], f32r, tag="tokc", name=f"tokc{e}_{cc}"
                            )
                            nc.gpsimd.dma_start(
                                tkc[:],
                                tok[e].rearrange(
                                    "(p c j) d -> c p j d", c=N_CHUNKS, p=P
                                )[cc],
                            )
                            tokn_chunks.append(tkc)
                    # w1[e]: [128 d, 512 hd] natural; cast-round to f32r in DMA
                    w1_sb = w_pool.tile([P, H], f32r, tag="w1")
                    nc.gpsimd.dma_start(w1_sb[:], w1[e])
                    # w2[e]: [512 hd, 128 o] -> 4 k-tiles [128, 128] on partitions
                    w2_sb = w_pool.tile([P, H_TILES, O], f32r, tag="w2")
                    nc.gpsimd.dma_start(
                        w2_sb[:], w2[e].rearrange("(k p) o -> p k o", p=P)
                    )
                    if not chunk_this:
                        tokn_full = tokn_pool.tile([P, N_BLKS, D], f32r, tag="tokf")
                        nc.gpsimd.dma_start(
                            tokn_full[:], tok[e].rearrange("(p m) d -> p m d", p=P)
                        )
                    # output staging, same layout as tokn
                    if not C.get("chunked_out"):
                        outn = outn_pool.tile([P, N_BLKS, O], f32)

                    for c in range(N_CHUNKS):
                        if chunk_this:
                            blk = lambda j, _t=tokn_chunks[c]: _t[:, j]
                        else:
                            blk = lambda j: tokn_full[:, c * BLKS_PER_CHUNK + j]
                        # transpose 4 token blocks into one psum tile -> tokT [d, 512 t]
                        pt = pt_pool.tile([P, T_CHUNK], f32r)
                        for j in range(BLKS_PER_CHUNK):
                            nc.tensor.transpose(
                                pt[:, j * P : (j + 1) * P], blk(j), ident[:]
                            )
                        tokt = tokt_pool.tile([P, T_CHUNK], f32r)
                        nc.vector.tensor_copy(tokt[:], pt[:])

                        # GEMM1 + GELU: hT[hd_tile] = gelu(w1_slice.T @ tokT)
                        ht_slices = []
                        if C.get("gelu_quad"):
                            ph = ph_pool.tile([P, H_TILES, T_CHUNK], f32)
                            for hd in range(H_TILES):
                                nc.tensor.matmul(
                                    ph[:, hd],
                                    w1_sb[:, hd * P : (hd + 1) * P],
                                    tokt[:],
                                    start=True,
                                    stop=True,
                                )
                            ht = h_pool.tile([P, H_TILES, T_CHUNK], f32r, tag="ht")
                            nc.scalar.activation(ht[:], ph[:], GELU)
                            ht_slices = [ht[:, hd] for hd in range(H_TILES)]
                        elif C["gelu_pair"]:
                            for hp in range(H_TILES // 2):
                                ph = ph_pool.tile([P, 2, T_CHUNK], f32)
                                for k in range(2):
                                    hd = hp * 2 + k
                                    nc.tensor.matmul(
                                        ph[:, k],
                                        w1_sb[:, hd * P : (hd + 1) * P],
                                        tokt[:],
                                        start=True,
                                        stop=True,
                                    )
                                ht = h_pool.tile([P, 2, T_CHUNK], f32r, tag="ht")
                                nc.scalar.activation(ht[:], ph[:], GELU)
                                ht_slices.extend([ht[:, 0], ht[:, 1]])
                        else:
                            for hd in range(H_TILES):
                                ph = ph_pool.tile([P, T_CHUNK], f32)
                                nc.tensor.matmul(
                                    ph[:],
                                    w1_sb[:, hd * P : (hd + 1) * P],
                                    tokt[:],
                                    start=True,
                                    stop=True,
                                )
                                ht = h_pool.tile([P, T_CHUNK], f32r, tag="ht")
                                nc.scalar.activation(ht[:], ph[:], GELU)
                                ht_slices.append(ht[:])

                        # GEMM2: outT[o, t] = sum_hd w2_tile.T @ hT[hd]
                        po = po_pool.tile([P, T_CHUNK], f32)
                        for hd in range(H_TILES):
                            nc.tensor.matmul(
                                po[:],
                                w2_sb[:, hd],
                                ht_slices[hd],
                                start=(hd == 0),
                                stop=(hd == H_TILES - 1),
                            )
                        osb = osb_pool.tile([P, T_CHUNK], f32r)
                        if C.get("osb_alt") and c % 2 == 1:
                            nc.scalar.copy(osb[:], po[:])
                        else:
                            nc.vector.tensor_copy(osb[:], po[:])

                        # transpose back: [o, t] -> [t, o] per 128-token block
                        pot = pot_pool.tile([P, T_CHUNK], f32r)
                        for j in range(BLKS_PER_CHUNK):
                            nc.tensor.transpose(
                                pot[:, j * P : (j + 1) * P],
                                osb[:, j * P : (j + 1) * P],
                                ident[:],
                            )
                        if C.get("chunked_out"):
                            oc = outn_pool.tile([P, BLKS_PER_CHUNK, O], f32, tag="oc")
                            nc.vector.tensor_copy(
                                oc[:],
                                pot[:].rearrange("p (j o) -> p j o", j=BLKS_PER_CHUNK),
                            )
                            nc.sync.dma_start(
                                out[e].rearrange(
                                    "(p c j) o -> c p j o", c=N_CHUNKS, p=P
                                )[c],
                                oc[:],
                            )
                        else:
                            nc.vector.tensor_copy(
                                outn[:, c * BLKS_PER_CHUNK : (c + 1) * BLKS_PER_CHUNK],
                                pot[:].rearrange("p (j o) -> p j o", j=BLKS_PER_CHUNK),
                            )

                    if not C.get("chunked_out"):
                        nc.sync.dma_start(
                            out[e].rearrange("(p m) o -> p m o", p=P), outn[:]
                        )

            def body_swpipe(_iv=None):
                """Software-pipelined emission: next chunk's token transposes are
                interleaved between this chunk's matmuls so transpose weight
                loads hide under matmul streaming (LDW is per-matmul for 4-byte
                stationaries and the cost model does not show this)."""
                NG = E_PER_CORE * N_CHUNKS  # 64 global chunks
                state = {}  # e -> (w1_sb, w2_sb, tok_tiles)

                def setup(e):
                    w1_sb = w_pool.tile([P, H], f32r, tag="w1", name=f"w1s{e}")
                    nc.gpsimd.dma_start(w1_sb[:], w1[e])
                    w2_sb = w_pool.tile([P, H_TILES, O], f32r, tag="w2", name=f"w2s{e}")
                    nc.gpsimd.dma_start(
                        w2_sb[:], w2[e].rearrange("(k p) o -> p k o", p=P)
                    )
                    if e == 0:
                        toks = []
                        for c in range(N_CHUNKS):
                            tk = tokn_pool.tile([P, BLKS_PER_CHUNK, D], f32r, tag="tokc", name=f"tokc{c}")
                            nc.gpsimd.dma_start(
                                tk[:],
                                tok[e].rearrange(
                                    "(p c j) d -> c p j d", c=N_CHUNKS, p=P
                                )[c],
                            )
                            toks.append(tk)
                    else:
                        tf = tokn_pool.tile([P, N_BLKS, D], f32r, tag="tokf", name=f"tokf{e}")
                        nc.gpsimd.dma_start(
                            tf[:], tok[e].rearrange("(p m) d -> p m d", p=P)
                        )
                        toks = tf
                    state[e] = (w1_sb, w2_sb, toks)

                def blk(g, j):
                    e, c = divmod(g, N_CHUNKS)
                    toks = state[e][2]
                    if isinstance(toks, list):
                        return toks[c][:, j]
                    return toks[:, c * BLKS_PER_CHUNK + j]

                pts = {}
                tokts = {}
                hts = {}
                pos = {}
                osbs = {}
                pots = {}

                def tin(g, j):
                    if j == 0:
                        pts[g] = pt_pool.tile([P, T_CHUNK], f32r, tag="pt", name=f"pt{g}")
                    nc.tensor.transpose(
                        pts[g][:, j * P : (j + 1) * P], blk(g, j), ident[:]
                    )

                def tout(g, j):
                    if j == 0:
                        pots[g] = pot_pool.tile([P, T_CHUNK], f32r, tag="pot", name=f"pot{g}")
                    nc.tensor.transpose(
                        pots[g][:, j * P : (j + 1) * P],
                        osbs[g][:, j * P : (j + 1) * P],
                        ident[:],
                    )

                def drain_out(g):
                    e, c = divmod(g, N_CHUNKS)
                    oc = outn_pool.tile([P, BLKS_PER_CHUNK, O], f32, tag="oc", name=f"oc{g}")
                    nc.vector.tensor_copy(
                        oc[:],
                        pots.pop(g)[:].rearrange("p (j o) -> p j o", j=BLKS_PER_CHUNK),
                    )
                    nc.sync.dma_start(
                        out[e].rearrange("(p c j) o -> c p j o", c=N_CHUNKS, p=P)[c],
                        oc[:],
                    )

                setup(0)
                for j in range(BLKS_PER_CHUNK):
                    tin(0, j)

                for g in range(NG):
                    e, c = divmod(g, N_CHUNKS)
                    if c == 2 and e + 1 < E_PER_CORE:
                        setup(e + 1)
                    w1_sb, w2_sb, _ = state[e]

                    tokts[g] = tokt_pool.tile([P, T_CHUNK], f32r, tag="tokt", name=f"tokt{g}")
                    nc.vector.tensor_copy(tokts[g][:], pts.pop(g)[:])

                    # MM1s interleaved with previous chunk's out-transposes
                    ht_slices = []
                    ph = None
                    for hd in range(H_TILES):
                        if hd % 2 == 0:
                            ph = ph_pool.tile([P, 2, T_CHUNK], f32, tag="ph", name=f"ph{g}_{hd}")
                        nc.tensor.matmul(
                            ph[:, hd % 2],
                            w1_sb[:, hd * P : (hd + 1) * P],
                            tokts[g][:],
                            start=True,
                            stop=True,
                        )
                        if g >= 1:
                            tout(g - 1, hd)
                        if hd % 2 == 1:
                            ht = h_pool.tile([P, 2, T_CHUNK], f32r, tag="ht", name=f"ht{g}_{hd}")
                            nc.scalar.activation(ht[:], ph[:], GELU)
                            ht_slices.extend([ht[:, 0], ht[:, 1]])
                    hts[g] = ht_slices
                    if g >= 1:
                        drain_out(g - 1)

                    # MM2s interleaved with next chunk's in-transposes
                    pos[g] = po_pool.tile([P, T_CHUNK], f32, tag="po", name=f"po{g}")
                    for hd in range(H_TILES):
                        nc.tensor.matmul(
                            pos[g][:],
                            w2_sb[:, hd],
                            hts[g][hd],
                            start=(hd == 0),
                            stop=(hd == H_TILES - 1),
                        )
                        if g + 1 < NG:
                            tin(g + 1, hd)
                    osbs[g] = osb_pool.tile([P, T_CHUNK], f32r, tag="osb", name=f"osb{g}")
                    nc.vector.tensor_copy(osbs[g][:], pos.pop(g)[:])
                    tokts.pop(g)

                for j in range(BLKS_PER_CHUNK):
                    tout(NG - 1, j)
                drain_out(NG - 1)

            chosen = body_swpipe if C.get("sw_pipe") else body
            if loop == 1:
                chosen()
            else:
                with tc.For_i(0, loop, 1) as _i:
                    chosen(_i)

    nc.compile()
    return nc


def _get_nc(loop=1, cfg=None):
    key = ("nc", loop, tuple(sorted((cfg or {}).items())))
    if key not in _CACHE:
        _CACHE[key] = _build(loop, cfg)
    return _CACHE[key]


def kernel(group_token, weights1, weights2):
    from concourse.bass_utils import run_bass_kernel_spmd

    group_token = np.ascontiguousarray(np.asarray(group_token, dtype=np.float32))
    weights1 = np.ascontiguousarray(np.asarray(weights1, dtype=np.float32))
    weights2 = np.ascontiguousarray(np.asarray(weights2, dtype=np.float32))

    nc = _get_nc()
    in_maps = []
    for c in range(NUM_CORES):
        sl = slice(c * E_PER_CORE, (c + 1) * E_PER_CORE)
        in_maps.append(
            {
                "group_token": np.ascontiguousarray(group_token[sl]),
                "weights1": np.ascontiguousarray(weights1[sl]),
                "weights2": np.ascontiguousarray(weights2[sl]),
            }
        )

    res = run_bass_kernel_spmd(nc, in_maps, core_ids=list(range(NUM_CORES)))
    _CACHE["last_results"] = res
    return np.concatenate([r["out"] for r in res.results], axis=0)



# revision 4
# speedup vs baseline: 2.2192x; 2.2192x over previous
"""Expert-parallel grouped GEMM (MoE) kernel for Trainium2.

Problem: out[e] = gelu(tok[e] @ w1[e]) @ w2[e]  per expert e.
  tok: [128, 2048, 128] f32, w1: [128, 128, 512] f32, w2: [128, 512, 128] f32.

Sharding: expert-parallel across 8 NeuronCores, 16 experts per core, no
cross-core communication. Each core runs the same Bass program on its own
expert slice (SPMD), the host concatenates the per-core outputs.

Per-core dataflow (v3):
  - tokens SWDGE-cast f32->bf16 on load, [128 p, 16 m, 128 d] (token t = p*16+m)
  - token transpose to [d, t]: one batched DMA-transpose (X-bar) per expert
    (cfg tok_path="dmat"), or PE transposes + DVE copies (cfg "pe")
  - GEMM1 on PE: w1 bf16 stationary (FWL), tokT moving, N=512 full rate
  - GELU on ACT in groups of `gelu_group` psum banks per instruction
    (amortizes the per-instruction fixed overhead), writes one big bf16
    SBUF tile hsb [128, 4*2048] per expert
  - GEMM2 "direct": stationary = hT 128-token block, moving = w2 tile, psum
    accumulates [t, o] directly -> no output transposes, single DVE drain copy
  - batched per-expert store [128 p, 16 m, 128 o] (HWDGE)
"""

import numpy as np

NUM_CORES = 8
E_TOTAL = 128
E_PER_CORE = E_TOTAL // NUM_CORES  # 16
T = 2048
D = 128
H = 512
O = 128
P = 128

N_BLKS = T // P  # 16 token blocks per expert
N_CHUNKS = 4
BLKS_PER_CHUNK = N_BLKS // N_CHUNKS  # 4
T_CHUNK = T // N_CHUNKS  # 512
H_TILES = H // P  # 4

_CACHE = {}


DEFAULT_CFG = dict(
    tok_path="dmat",  # "dmat": batched DMA-transpose; "pe": PE transposes
    g2="direct",  # "direct": hT stationary, [t,o] psum; "classic": w2 stationary
    gelu_group=3,  # psum banks per ACT gelu instruction
    load_ahead=2,
    ph_bufs=2,
    po_bufs=2,
    pt_bufs=2,
    pot_bufs=2,
    osb_bufs=2,
    tokn_bufs=3,
    tokt_bufs=3,
    h_bufs=2,
    outsb_bufs=3,
    w_bufs=3,
)


def _build(loop=1, cfg=None):
    import concourse.bacc as bacc
    import concourse.mybir as mybir
    import concourse.tile as tile
    from concourse.masks import make_identity

    f32 = mybir.dt.float32
    bf16 = mybir.dt.bfloat16
    GELU = mybir.ActivationFunctionType.Gelu
    C = dict(DEFAULT_CFG)
    if cfg:
        C.update(cfg)

    E = E_PER_CORE
    GG = C["gelu_group"]
    N_TILES = H_TILES * N_CHUNKS  # 16 (hd, c) psum tiles per expert

    from contextlib import ExitStack

    nc = bacc.Bacc(
        "TRN2",
        target_bir_lowering=False,
        debug=False,
        num_devices=NUM_CORES,
    )

    tok = nc.dram_tensor("group_token", [E, T, D], f32, kind="ExternalInput").ap()
    w1 = nc.dram_tensor("weights1", [E, D, H], f32, kind="ExternalInput").ap()
    w2 = nc.dram_tensor("weights2", [E, H, O], f32, kind="ExternalInput").ap()
    out = nc.dram_tensor("out", [E, T, O], f32, kind="ExternalOutput").ap()

    with tile.TileContext(nc) as tc:
        with ExitStack() as stack:
            const_pool = stack.enter_context(tc.tile_pool(name="const", bufs=1))
            w_pool = stack.enter_context(tc.tile_pool(name="weights", bufs=C["w_bufs"]))
            tokn_pool = stack.enter_context(tc.tile_pool(name="tokn", bufs=C["tokn_bufs"]))
            tokt_pool = stack.enter_context(tc.tile_pool(name="tokt", bufs=C["tokt_bufs"]))
            h_pool = stack.enter_context(tc.tile_pool(name="hts", bufs=C["h_bufs"]))
            outsb_pool = stack.enter_context(tc.tile_pool(name="outsb", bufs=C["outsb_bufs"]))
            ph_pool = stack.enter_context(tc.tile_pool(name="ph", bufs=C["ph_bufs"], space="PSUM"))
            po_pool = stack.enter_context(tc.tile_pool(name="po", bufs=C["po_bufs"], space="PSUM"))

            need_ident = C["tok_path"] == "pe" or C["g2"] == "classic"
            if need_ident:
                ident_f32 = const_pool.tile([P, P], f32)
                make_identity(nc, ident_f32)
                ident = const_pool.tile([P, P], bf16)
                nc.vector.tensor_copy(ident[:], ident_f32[:])

            if C["tok_path"] == "pe":
                pt_pool = stack.enter_context(
                    tc.tile_pool(name="pt", bufs=C["pt_bufs"], space="PSUM")
                )
            if C["g2"] == "classic":
                pot_pool = stack.enter_context(
                    tc.tile_pool(name="pot", bufs=C["pot_bufs"], space="PSUM")
                )
                osb_pool = stack.enter_context(
                    tc.tile_pool(name="osb", bufs=C["osb_bufs"])
                )

            def body(_iv=None):
                tokn = {}
                tokT = {}
                hsb = {}

                def load(e):
                    tkn = tokn_pool.tile([P, N_BLKS, D], bf16, tag="tokn", name=f"tokn{e}")
                    nc.gpsimd.dma_start(
                        tkn[:], tok[e].rearrange("(p m) d -> p m d", p=P)
                    )
                    tokn[e] = tkn
                    w1bf = w_pool.tile([P, H], bf16, tag="w1", name=f"w1b{e}")
                    nc.gpsimd.dma_start(w1bf[:], w1[e])
                    w2bf = w_pool.tile([P, H_TILES, O], bf16, tag="w2", name=f"w2b{e}")
                    nc.gpsimd.dma_start(
                        w2bf[:], w2[e].rearrange("(k p) o -> p k o", p=P)
                    )
                    tokn[e, "w"] = (w1bf, w2bf)

                def tin(e):
                    # tokT[d, m, p]: token t = p*16 + m lives at column m*128+p
                    tt = tokt_pool.tile([P, N_BLKS, P], bf16, tag="tokt", name=f"tokt{e}")
                    if C["tok_path"] == "dmat":
                        nc.sync.dma_start(
                            tt[:],
                            tokn[e][:].rearrange("p m d -> p (m d)"),
                            transpose=True,
                        )
                    else:
                        for c in range(N_CHUNKS):
                            pt = pt_pool.tile([P, T_CHUNK], bf16, tag="pt")
                            for j in range(BLKS_PER_CHUNK):
                                nc.tensor.transpose(
                                    pt[:, j * P : (j + 1) * P],
                                    tokn[e][:, c * BLKS_PER_CHUNK + j],
                                    ident[:],
                                )
                            nc.vector.tensor_copy(
                                tt[:, c * BLKS_PER_CHUNK : (c + 1) * BLKS_PER_CHUNK],
                                pt[:].rearrange("p (m q) -> p m q", m=BLKS_PER_CHUNK),
                            )
                    tokT[e] = tt

                def g1(e):
                    w1bf, _ = tokn[e, "w"]
                    # hsb columns: flat = (hd*4 + c)*512 + i
                    hs = h_pool.tile([P, H_TILES * T], bf16, tag="hsb", name=f"hsb{e}")
                    hsb[e] = hs
                    tt = tokT[e][:].rearrange("p m q -> p (m q)")
                    ph = None
                    base = 0
                    for flat in range(N_TILES):
                        hd, c = divmod(flat, N_CHUNKS)
                        i = flat % GG
                        if i == 0:
                            gsz = min(GG, N_TILES - flat)
                            ph = ph_pool.tile([P, gsz, T_CHUNK], f32, tag="ph")
                            base = flat
                        nc.tensor.matmul(
                            ph[:, i],
                            w1bf[:, hd * P : (hd + 1) * P],
                            tt[:, c * T_CHUNK : (c + 1) * T_CHUNK],
                            start=True,
                            stop=True,
                        )
                        if i == gsz - 1:
                            nc.scalar.activation(
                                hs[:, base * T_CHUNK : (flat + 1) * T_CHUNK],
                                ph[:].rearrange("p g q -> p (g q)"),
                                GELU,
                            )

                def g2_direct(e):
                    _, w2bf = tokn[e, "w"]
                    hs = hsb[e]
                    osb = outsb_pool.tile([P, N_BLKS, O], f32, tag="outsb", name=f"osb{e}")
                    for c in range(N_CHUNKS):
                        po = po_pool.tile([P, BLKS_PER_CHUNK, O], f32, tag="po")
                        for j in range(BLKS_PER_CHUNK):
                            m = c * BLKS_PER_CHUNK + j
                            for hd in range(H_TILES):
                                nc.tensor.matmul(
                                    po[:, j],
                                    hs[:, (hd * N_BLKS + m) * P : (hd * N_BLKS + m + 1) * P],
                                    w2bf[:, hd],
                                    start=(hd == 0),
                                    stop=(hd == H_TILES - 1),
                                )
                        nc.vector.tensor_copy(
                            osb[:, c * BLKS_PER_CHUNK : (c + 1) * BLKS_PER_CHUNK],
                            po[:],
                        )
                    return osb

                def g2_classic(e):
                    _, w2bf = tokn[e, "w"]
                    hs = hsb[e]
                    osb_out = outsb_pool.tile(
                        [P, N_BLKS, O], f32, tag="outsb", name=f"osb{e}"
                    )
                    for c in range(N_CHUNKS):
                        po = po_pool.tile([P, T_CHUNK], f32, tag="po")
                        for hd in range(H_TILES):
                            nc.tensor.matmul(
                                po[:],
                                w2bf[:, hd],
                                hs[:, (hd * N_CHUNKS + c) * T_CHUNK : (hd * N_CHUNKS + c + 1) * T_CHUNK],
                                start=(hd == 0),
                                stop=(hd == H_TILES - 1),
                            )
                        ob = osb_pool.tile([P, T_CHUNK], bf16, tag="ob")
                        nc.vector.tensor_copy(ob[:], po[:])
                        pot = pot_pool.tile([P, T_CHUNK], bf16, tag="pot")
                        for j in range(BLKS_PER_CHUNK):
                            nc.tensor.transpose(
                                pot[:, j * P : (j + 1) * P],
                                ob[:, j * P : (j + 1) * P],
                                ident[:],
                            )
                        nc.vector.tensor_copy(
                            osb_out[:, c * BLKS_PER_CHUNK : (c + 1) * BLKS_PER_CHUNK],
                            pot[:].rearrange("p (m q) -> p m q", m=BLKS_PER_CHUNK),
                        )
                    return osb_out

                g2 = g2_direct if C["g2"] == "direct" else g2_classic

                def store(e, osb):
                    nc.sync.dma_start(
                        out[e].rearrange("(p m) o -> p m o", p=P), osb[:]
                    )

                LA = C["load_ahead"]
                for e in range(min(LA, E)):
                    load(e)
                tin(0)
                pending = {}
                for e in range(E):
                    if e + LA < E:
                        load(e + LA)
                    if e + 1 < E:
                        tin(e + 1)
                    g1(e)
                    if e > 0:
                        pending[e - 1] = g2(e - 1)
                        store(e - 1, pending.pop(e - 1))
                pending[E - 1] = g2(E - 1)
                store(E - 1, pending.pop(E - 1))

            if loop == 1:
                body()
            else:
                with tc.For_i(0, loop, 1) as _i:
                    body(_i)

    nc.compile()
    return nc


def _get_nc(loop=1, cfg=None):
    key = ("nc", loop, tuple(sorted((cfg or {}).items())))
    if key not in _CACHE:
        _CACHE[key] = _build(loop, cfg)
    return _CACHE[key]


def kernel(group_token, weights1, weights2):
    from concourse.bass_utils import run_bass_kernel_spmd

    group_token = np.ascontiguousarray(np.asarray(group_token, dtype=np.float32))
    weights1 = np.ascontiguousarray(np.asarray(weights1, dtype=np.float32))
    weights2 = np.ascontiguousarray(np.asarray(weights2, dtype=np.float32))

    nc = _get_nc()
    in_maps = []
    for c in range(NUM_CORES):
        sl = slice(c * E_PER_CORE, (c + 1) * E_PER_CORE)
        in_maps.append(
            {
                "group_token": np.ascontiguousarray(group_token[sl]),
                "weights1": np.ascontiguousarray(weights1[sl]),
                "weights2": np.ascontiguousarray(weights2[sl]),
            }
        )

    res = run_bass_kernel_spmd(nc, in_maps, core_ids=list(range(NUM_CORES)))
    _CACHE["last_results"] = res
    return np.concatenate([r["out"] for r in res.results], axis=0)
